# revision 44
# baseline (speedup 1.0000x reference)
"""Trainium2 Bass kernel for a GNN message-passing layer.

reference semantics (jax):
    src, dst = edge_index
    messages   = silu(concat(nodes[src], edge_features) @ mw1 + mb1)    # [E, D]
    aggregated = segment_sum(messages, dst, N)                          # [N, D]
    updated    = silu(concat(nodes, aggregated) @ uw1 + ub1) @ uw2 + ub2
    out        = nodes + updated

Distribution: destination-node-tile partition across 8 cores with greedy
load balancing (tiles assigned to cores by descending edge count). Nodes
and MLP weights are replicated; each core aggregates exactly the edges
landing in its tiles and runs the update MLP on them. No collectives.

Host-side work is limited to layout transforms of inputs (slicing,
padding, bf16 rounding, permutation of nodes/edge_features rows into
slot order, per-tile 128x128 block transposes) — no float arithmetic.

Per edge slot the device receives nodes[src] and edge_features rows
(interleaved per edge tile, transposed, bf16). The message MLP is two
accumulating matmuls per edge tile (contraction over the two 128-dim
halves of mw1) — no on-device gather at all (a prior version gathered
X = nodes@mw1[:D] per edge; SWDGE descriptor generation at ~6ns/row
made that a ~600us floor).

Device pipeline per core:
  Per node tile: one DMA of the packed bf16 [nodes[src] | ef]^T stream;
  per 4-edge-tile chunk: 8 matmuls into one PSUM group, one DVE add of
  mb1 (broadcast), one SiLU; per chunk one batched one-hot build
  (GpSimd) and 4 scatter matmuls accumulating agg^T [d, j] in PSUM.
  Every 4 node tiles, one update-MLP group (transposed space) runs
  interleaved: silu(nodes@uw1[:D] + agg@uw1[D:] + ub1) @ uw2 + ub2 +
  residual, transpose back, store.
"""

import math
import sys

sys.path.insert(0, "/opt/trn_rl_repo")

import ml_dtypes
import numpy as np

import concourse.bacc as bacc
import concourse.mybir as mybir
import concourse.tile as tile
from concourse import bass_utils

P = 128
C = 8  # cores

F32 = mybir.dt.float32
BF16 = mybir.dt.bfloat16
FP8 = mybir.dt.float8e4
AF = mybir.ActivationFunctionType
OP = mybir.AluOpType
BF = ml_dtypes.bfloat16
F8 = mybir.dt.np(FP8)
WSC = 32.0  # fp8 weight scale (power of two; undone in the silu scale)


def _tileT(a):
    """[R*P, D] -> [R*D, P] with each 128-row block transposed."""
    R = a.shape[0] // P
    return np.ascontiguousarray(
        a.reshape(R, P, a.shape[1]).transpose(0, 2, 1)
    ).reshape(R * a.shape[1], P)


def _host_prep(nodes, edge_index, edge_features, ntiles_pc):
    """Bucket edges by destination node tile, balance tiles over cores,
    pack [nodes[src] | ef] slot streams."""
    N, D = nodes.shape
    E = edge_index.shape[1]
    N2 = ntiles_pc * P * C
    ntiles = N2 // P

    src = edge_index[0].astype(np.int64)
    dst = edge_index[1].astype(np.int64)
    order = np.argsort(dst // P, kind="stable").astype(np.int64)
    ds = dst[order]
    ss = src[order]

    tileid = ds // P
    counts = np.bincount(tileid, minlength=ntiles)

    # greedy balance: biggest tiles first onto the least-loaded core
    assign = [[] for _ in range(C)]  # global tile ids per core, local order
    loads = np.zeros(C, np.int64)
    for g in np.argsort(-counts, kind="stable"):
        cands = [c for c in range(C) if len(assign[c]) < ntiles_pc]
        c = min(cands, key=lambda c: (loads[c], len(assign[c])))
        assign[c].append(int(g))
        loads[c] += counts[g]
    # per local position, tile counts across cores -> shared trip counts
    pos_counts = np.zeros((C, ntiles_pc), np.int64)
    for c in range(C):
        for p, g in enumerate(assign[c]):
            pos_counts[c, p] = counts[g]
    ktot = [int(math.ceil(pos_counts[:, p].max() / P)) for p in range(ntiles_pc)]
    offs = np.zeros(ntiles_pc + 1, np.int64)
    np.cumsum(ktot, out=offs[1:])
    SL = int(offs[-1]) * P  # packed slots per core

    # map: global tile -> (core, local pos)
    t2cp = np.zeros((ntiles, 2), np.int64)
    for c in range(C):
        for p, g in enumerate(assign[c]):
            t2cp[g] = (c, p)

    tile_start = np.zeros(ntiles + 1, np.int64)
    np.cumsum(counts, out=tile_start[1:])
    rank = np.arange(E, dtype=np.int64) - tile_start[tileid]
    core = t2cp[tileid, 0]
    pos = t2cp[tileid, 1]
    slot = offs[pos] * P + rank

    dstoff = np.full((C, SL), -1.0, np.float32)
    dstoff[core, slot] = (ds - tileid * P).astype(np.float32)
    esrc = np.full((C, SL), -1, np.int64)  # edge id feeding each slot
    esrc[core, slot] = order

    ef16 = edge_features.astype(F8)
    n16 = nodes.astype(F8)
    jj = np.arange(P, dtype=np.float32)
    per_core = []
    for c in range(C):
        valid = esrc[c] >= 0
        a = np.zeros((SL, D), F8)  # nodes[src] rows
        b = np.zeros((SL, D), F8)  # ef rows
        eidx = esrc[c][valid]
        a[valid] = n16[src[eidx]]
        b[valid] = ef16[eidx]
        # scatter one-hot rows: oh[e, j] = (dstoff[e] == j); pads (-1) -> 0
        oh = (dstoff[c][:, None] == jj[None, :]).astype(F8)  # [SL, P]
        # interleave per edge tile: [nsrcT_k | efT_k | oh_k] blocks
        aT = a.reshape(SL // P, P, D).transpose(0, 2, 1)  # [K, D, P]
        bT = b.reshape(SL // P, P, D).transpose(0, 2, 1)
        ohK = oh.reshape(SL // P, P, P)  # natural [e, j] rows
        st = np.ascontiguousarray(
            np.stack([aT, bT, ohK], axis=1).reshape((SL // P) * 3 * D, P)
        )
        per_core.append(dict(st=st))
    return ktot, assign, per_core


def build_program(N2, D, ntiles_pc, ktot):
    """Build the SPMD Bass program (identical across cores)."""
    assert D == P
    ktot = list(ktot)
    offs = [0]
    for t in range(ntiles_pc):
        offs.append(offs[-1] + ktot[t])
    SL = offs[-1] * P

    nc = bacc.Bacc("TRN2", target_bir_lowering=False, debug=False, num_devices=C)
    NP_ = ntiles_pc * P

    d = lambda name, shape, dt=F32, kind="ExternalInput": nc.dram_tensor(
        name, shape, dt, kind=kind
    ).ap()

    st_d = d("st", [(SL // P) * 3 * D, P], FP8)
    ownT_d = d("own_nodesT", [ntiles_pc * D, P])
    wtb = d("wtb", [D, 2 * D])  # [wt*WSC | wb*WSC]
    mb8 = d("mb8", [P, 8 * D])  # mb1*WSC tiled
    ua = d("ua", [D, D])
    ub = d("ub", [D, D])
    uw2 = d("uw2", [D, D])
    ub1c = d("ub1c", [P, 1])
    ub2c = d("ub2c", [P, 1])
    ident = d("ident", [P, P])
    out = d("out_own", [NP_, D], kind="ExternalOutput")

    with tile.TileContext(nc) as tc:
        with (
            tc.tile_pool(name="const", bufs=1) as cp,
            tc.tile_pool(name="sb", bufs=5) as sb,
            tc.tile_pool(name="big", bufs=3) as bigp,
            tc.tile_pool(name="psum2", bufs=1, space="PSUM") as pp,
            tc.tile_pool(name="psumM", bufs=4, space="PSUM") as ppm,
            tc.tile_pool(name="psumA", bufs=3, space="PSUM") as ppa,
        ):
            def load_const(ap, shape, dt=F32):
                t = cp.tile(shape, dt, tag=ap.name)
                nc.sync.dma_start(out=t[:], in_=ap[:])
                return t

            wtb_s = load_const(wtb, [D, 2 * D])
            mb8_s = load_const(mb8, [P, 8 * D])
            ua_s = load_const(ua, [D, D])
            ub_s = load_const(ub, [D, D])
            uw2_s = load_const(uw2, [D, D])
            ub1_s = load_const(ub1c, [P, 1])
            ub2_s = load_const(ub2c, [P, 1])
            id_s = load_const(ident, [P, P])
            aggT_all = cp.tile([P, ntiles_pc * D], F32, tag="aggT_all")
            wtb8 = cp.tile([D, 2 * D], FP8, tag="wtb8")
            nc.vector.tensor_copy(out=wtb8[:], in_=wtb_s[:])
            wtb8_r = wtb8[:].rearrange("p (two f) -> p two f", two=2)

            def update_group(g):
                """Stage 3 for node tiles [4g, 4g+4): update MLP + residual."""
                gw = min(4, ntiles_pc - g * 4)
                W = gw * P
                g0 = g * 4
                ownT = sb.tile([P, 4 * P], F32, tag="ownT")
                nc.sync.dma_start(
                    out=ownT[:, :W].rearrange("p (j n) -> p j n", n=P),
                    in_=ownT_d[g0 * D : (g0 + gw) * D, :].rearrange(
                        "(j d) n -> d j n", d=D
                    ),
                )
                ph = pp.tile([P, 4 * P], F32, tag="ph")
                nc.tensor.matmul(
                    out=ph[:, :W], lhsT=ua_s[:], rhs=ownT[:, :W], start=True,
                    stop=False,
                )
                nc.tensor.matmul(
                    out=ph[:, :W],
                    lhsT=ub_s[:],
                    rhs=aggT_all[:, g0 * D : g0 * D + W],
                    start=False,
                    stop=True,
                )
                hT = sb.tile([P, 4 * P], F32, tag="hT")
                nc.scalar.activation(
                    out=hT[:, :W], in_=ph[:, :W], func=AF.Silu, bias=ub1_s[:, :1]
                )
                po = pp.tile([P, 4 * P], F32, tag="ph")
                nc.tensor.matmul(
                    out=po[:, :W], lhsT=uw2_s[:], rhs=hT[:, :W], start=True, stop=True
                )
                oT = sb.tile([P, 4 * P], F32, tag="oT")
                nc.scalar.activation(
                    out=oT[:, :W], in_=po[:, :W], func=AF.Identity, bias=ub2_s[:, :1]
                )
                nc.vector.tensor_tensor(
                    out=oT[:, :W], in0=oT[:, :W], in1=ownT[:, :W], op=OP.add
                )
                pOut = pp.tile([P, 4 * P], F32, tag="ph")
                for j in range(gw):
                    nc.tensor.transpose(
                        out=pOut[:, j * P : (j + 1) * P],
                        in_=oT[:, j * P : (j + 1) * P],
                        identity=id_s[:],
                    )
                ot = sb.tile([P, 4 * P], F32, tag="ot")
                nc.vector.tensor_copy(out=ot[:, :W], in_=pOut[:, :W])
                nc.sync.dma_start(
                    out=out[g0 * P : (g0 + gw) * P, :].rearrange(
                        "(j p) d -> p j d", p=P
                    ),
                    in_=ot[:, :W].rearrange("p (j d) -> p j d", d=D),
                )

            # empty (pure-pad) tiles never write aggT_all; clear once
            nc.vector.memset(aggT_all[:], 0)

            for t in range(ntiles_pc):
                kt = ktot[t]
                if kt:
                    egT = bigp.tile([P, 3 * kt * D], FP8, tag="egT")
                    eng = nc.scalar if t % 2 == 0 else nc.sync
                    eng.dma_start(
                        out=egT[:].rearrange("p (k e) -> p k e", e=P),
                        in_=st_d[offs[t] * 3 * D : offs[t + 1] * 3 * D, :].rearrange(
                            "(k d) e -> d k e", d=D
                        ),
                    )
                    LAG = 3  # scatter mms trail msg mms by this many chunks
                    paggT = ppa.tile([P, D], F32, tag="paggT")

                    def scatter(k0, cw, msg):
                        for j in range(cw):
                            k = k0 + j
                            # aggT[d, j] += msg_k^T-contraction over e
                            nc.tensor.matmul(
                                out=paggT[:],
                                lhsT=msg[:, j * P : (j + 1) * P],
                                rhs=egT[:, ((k * 3) + 2) * D : ((k * 3) + 3) * D],
                                start=(k == 0),
                                stop=(k == kt - 1),
                            )

                    pend = []
                    for ci in range(math.ceil(kt / 4)):
                        k0 = ci * 4
                        cw = min(4, kt - k0)
                        W = cw * P
                        pmsg = ppm.tile([P, 4 * P], F32, tag="pmsg")
                        for j in range(cw):
                            o = (k0 + j) * 3
                            nc.tensor.matmul(
                                out=pmsg[:, j * P : (j + 1) * P],
                                lhsT=egT[:, o * D : (o + 2) * D].rearrange(
                                    "p (two e) -> p two e", two=2
                                ),
                                rhs=wtb8_r,
                                start=True,
                                stop=True,
                                perf_mode=mybir.MatmulPerfMode.DoubleRow,
                            )
                        nc.vector.tensor_tensor(
                            out=pmsg[:, :W],
                            in0=pmsg[:, :W],
                            in1=mb8_s[:, :W],
                            op=OP.add,
                        )
                        msg = sb.tile([P, 4 * P], BF16, tag="msg")
                        nc.scalar.activation(
                            out=msg[:, :W],
                            in_=pmsg[:, :W],
                            func=AF.Silu,
                            scale=1.0 / WSC,
                        )
                        pend.append((k0, cw, msg))
                        if len(pend) > LAG:
                            scatter(*pend.pop(0))
                    for args in pend:
                        scatter(*args)
                    nc.vector.tensor_copy(
                        out=aggT_all[:, t * D : (t + 1) * D], in_=paggT[:]
                    )
                if t % 4 == 3:
                    update_group(t // 4)
            if ntiles_pc % 4:
                update_group(ntiles_pc // 4)

    nc.compile()
    return nc


def _run(nc, in_maps, trace=False):
    return bass_utils.run_bass_kernel_spmd(
        nc, in_maps, core_ids=list(range(C)), trace=trace
    )


def make_in_maps(nodes, edge_index, edge_features, mw1, mb1, uw1, ub1, uw2, ub2,
                 ntiles_pc):
    N, D = nodes.shape
    NP_ = ntiles_pc * P
    N2 = NP_ * C
    ktot, assign, per_core = _host_prep(nodes, edge_index, edge_features, ntiles_pc)

    nodes_pad = np.zeros((N2, D), np.float32)
    nodes_pad[:N] = nodes
    ident = np.eye(P, dtype=np.float32)
    mb8 = np.broadcast_to(
        np.tile(mb1.astype(np.float32) * WSC, 8), (P, 8 * D)
    ).copy()

    shared = dict(
        wtb=np.concatenate(
            [mw1[:D] * WSC, mw1[D:] * WSC], axis=1
        ).astype(np.float32),
        mb8=mb8,
        ua=np.ascontiguousarray(uw1[:D], np.float32),
        ub=np.ascontiguousarray(uw1[D:], np.float32),
        uw2=np.ascontiguousarray(uw2, np.float32),
        ub1c=np.ascontiguousarray(ub1.reshape(D, 1), np.float32),
        ub2c=np.ascontiguousarray(ub2.reshape(D, 1), np.float32),
        ident=ident,
    )
    in_maps = []
    for c in range(C):
        m = dict(shared)
        own = np.concatenate(
            [nodes_pad[g * P : (g + 1) * P] for g in assign[c]], axis=0
        )
        m["own_nodesT"] = _tileT(np.ascontiguousarray(own))
        m["st"] = per_core[c]["st"]
        in_maps.append(m)
    return ktot, assign, in_maps


def kernel(nodes, edge_index, edge_features, mw1, mb1, uw1, ub1, uw2, ub2):
    nodes = np.asarray(nodes, np.float32)
    edge_index = np.asarray(edge_index, np.int32)
    edge_features = np.asarray(edge_features, np.float32)
    N, D = nodes.shape
    ntiles_pc = math.ceil(N / (C * P))
    ktot, assign, in_maps = make_in_maps(
        nodes, edge_index, edge_features, mw1, mb1, uw1, ub1, uw2, ub2, ntiles_pc
    )
    N2 = ntiles_pc * P * C
    nc = build_program(N2, D, ntiles_pc, ktot)
    res = _run(nc, in_maps)
    out_full = np.zeros((N2, D), np.float32)
    for c in range(C):
        oc = res.results[c]["out_own"]
        for p, g in enumerate(assign[c]):
            out_full[g * P : (g + 1) * P] = oc[p * P : (p + 1) * P]
    return out_full[:N].astype(np.float32)


if __name__ == "__main__":
    rng = np.random.default_rng(0)
    N, E, D = 4096, 16384, 128
    nodes = rng.standard_normal((N, D), dtype=np.float32)
    edge_index = rng.integers(0, N, (2, E)).astype(np.int32)
    ef = rng.standard_normal((E, D), dtype=np.float32)
    s2, s1 = 1 / np.sqrt(2 * D), 1 / np.sqrt(D)
    mw1 = rng.uniform(-s2, s2, (2 * D, D)).astype(np.float32)
    mb1 = rng.uniform(-s2, s2, D).astype(np.float32)
    uw1 = rng.uniform(-s2, s2, (2 * D, D)).astype(np.float32)
    ub1 = rng.uniform(-s2, s2, D).astype(np.float32)
    uw2 = rng.uniform(-s1, s1, (D, D)).astype(np.float32)
    ub2 = rng.uniform(-s1, s1, D).astype(np.float32)

    def silu(x):
        return x / (1 + np.exp(-x))

    def ref():
        src, dst = edge_index
        msg = silu(np.concatenate([nodes[src], ef], 1) @ mw1 + mb1)
        agg = np.zeros((N, D), np.float32)
        np.add.at(agg, dst, msg)
        upd = silu(np.concatenate([nodes, agg], 1) @ uw1 + ub1) @ uw2 + ub2
        return nodes + upd

    out = kernel(nodes, edge_index, ef, mw1, mb1, uw1, ub1, uw2, ub2)
    exp = ref()
    err = np.abs(out - exp).max() / np.abs(exp).max()
    print("tiny rel err:", err)


# revision 45
# speedup vs baseline: 1.2389x; 1.2389x over previous
"""Trainium2 Bass kernel for a GNN message-passing layer.

reference semantics (jax):
    src, dst = edge_index
    messages   = silu(concat(nodes[src], edge_features) @ mw1 + mb1)    # [E, D]
    aggregated = segment_sum(messages, dst, N)                          # [N, D]
    updated    = silu(concat(nodes, aggregated) @ uw1 + ub1) @ uw2 + ub2
    out        = nodes + updated

Distribution: destination-node-tile partition across 8 cores with greedy
load balancing (tiles assigned to cores by descending edge count). Nodes
and MLP weights are replicated; each core aggregates exactly the edges
landing in its tiles and runs the update MLP on them. No collectives.

Host-side work is limited to layout transforms of inputs (slicing,
padding, bf16 rounding, permutation of nodes/edge_features rows into
slot order, per-tile 128x128 block transposes) — no float arithmetic.

Per edge slot the device receives nodes[src] and edge_features rows
(interleaved per edge tile, transposed, bf16). The message MLP is two
accumulating matmuls per edge tile (contraction over the two 128-dim
halves of mw1) — no on-device gather at all (a prior version gathered
X = nodes@mw1[:D] per edge; SWDGE descriptor generation at ~6ns/row
made that a ~600us floor).

Device pipeline per core:
  Per node tile: one DMA of the packed bf16 [nodes[src] | ef]^T stream;
  per 4-edge-tile chunk: 8 matmuls into one PSUM group, one DVE add of
  mb1 (broadcast), one SiLU; per chunk one batched one-hot build
  (GpSimd) and 4 scatter matmuls accumulating agg^T [d, j] in PSUM.
  Every 4 node tiles, one update-MLP group (transposed space) runs
  interleaved: silu(nodes@uw1[:D] + agg@uw1[D:] + ub1) @ uw2 + ub2 +
  residual, transpose back, store.
"""

import math
import sys

sys.path.insert(0, "/opt/trn_rl_repo")

import ml_dtypes
import numpy as np

import concourse.bacc as bacc
import concourse.mybir as mybir
import concourse.tile as tile
from concourse import bass_utils

P = 128
C = 8  # cores

F32 = mybir.dt.float32
BF16 = mybir.dt.bfloat16
FP8 = mybir.dt.float8e4
AF = mybir.ActivationFunctionType
OP = mybir.AluOpType
BF = ml_dtypes.bfloat16
F8 = mybir.dt.np(FP8)
WSC = 32.0  # fp8 weight scale (power of two; undone in the silu scale)


def _tileT(a):
    """[R*P, D] -> [R*D, P] with each 128-row block transposed."""
    R = a.shape[0] // P
    return np.ascontiguousarray(
        a.reshape(R, P, a.shape[1]).transpose(0, 2, 1)
    ).reshape(R * a.shape[1], P)


def _host_prep(nodes, edge_index, edge_features, ntiles_pc):
    """Bucket edges by destination node tile, balance tiles over cores,
    pack [nodes[src] | ef] slot streams."""
    N, D = nodes.shape
    E = edge_index.shape[1]
    N2 = ntiles_pc * P * C
    ntiles = N2 // P

    src = edge_index[0].astype(np.int64)
    dst = edge_index[1].astype(np.int64)
    order = np.argsort(dst // P, kind="stable").astype(np.int64)
    ds = dst[order]
    ss = src[order]

    tileid = ds // P
    counts = np.bincount(tileid, minlength=ntiles)

    # greedy balance: biggest tiles first onto the least-loaded core
    assign = [[] for _ in range(C)]  # global tile ids per core, local order
    loads = np.zeros(C, np.int64)
    for g in np.argsort(-counts, kind="stable"):
        cands = [c for c in range(C) if len(assign[c]) < ntiles_pc]
        c = min(cands, key=lambda c: (loads[c], len(assign[c])))
        assign[c].append(int(g))
        loads[c] += counts[g]
    # per local position, tile counts across cores -> shared trip counts
    pos_counts = np.zeros((C, ntiles_pc), np.int64)
    for c in range(C):
        for p, g in enumerate(assign[c]):
            pos_counts[c, p] = counts[g]
    ktot = [int(math.ceil(pos_counts[:, p].max() / P)) for p in range(ntiles_pc)]
    offs = np.zeros(ntiles_pc + 1, np.int64)
    np.cumsum(ktot, out=offs[1:])
    SL = int(offs[-1]) * P  # packed slots per core

    # map: global tile -> (core, local pos)
    t2cp = np.zeros((ntiles, 2), np.int64)
    for c in range(C):
        for p, g in enumerate(assign[c]):
            t2cp[g] = (c, p)

    tile_start = np.zeros(ntiles + 1, np.int64)
    np.cumsum(counts, out=tile_start[1:])
    rank = np.arange(E, dtype=np.int64) - tile_start[tileid]
    core = t2cp[tileid, 0]
    pos = t2cp[tileid, 1]
    slot = offs[pos] * P + rank

    dstoff = np.full((C, SL), -1.0, np.float32)
    dstoff[core, slot] = (ds - tileid * P).astype(np.float32)
    esrc = np.full((C, SL), -1, np.int64)  # edge id feeding each slot
    esrc[core, slot] = order

    ef16 = edge_features.astype(F8)
    n16 = nodes.astype(F8)
    jj = np.arange(P, dtype=np.float32)
    per_core = []
    for c in range(C):
        valid = esrc[c] >= 0
        a = np.zeros((SL, D), F8)  # nodes[src] rows
        b = np.zeros((SL, D), F8)  # ef rows
        eidx = esrc[c][valid]
        a[valid] = n16[src[eidx]]
        b[valid] = ef16[eidx]
        # scatter one-hot rows: oh[e, j] = (dstoff[e] == j); pads (-1) -> 0
        oh = (dstoff[c][:, None] == jj[None, :]).astype(F8)  # [SL, P]
        # interleave per edge tile: [nsrcT_k | efT_k | oh_k] blocks
        aT = a.reshape(SL // P, P, D).transpose(0, 2, 1)  # [K, D, P]
        bT = b.reshape(SL // P, P, D).transpose(0, 2, 1)
        ohK = oh.reshape(SL // P, P, P)  # natural [e, j] rows
        st = np.ascontiguousarray(
            np.stack([aT, bT, ohK], axis=1).reshape((SL // P) * 3 * D, P)
        )
        per_core.append(dict(st=st))
    return ktot, assign, per_core


def build_program(N2, D, ntiles_pc, ktot):
    """Build the SPMD Bass program (identical across cores)."""
    assert D == P
    ktot = list(ktot)
    offs = [0]
    for t in range(ntiles_pc):
        offs.append(offs[-1] + ktot[t])
    SL = offs[-1] * P

    nc = bacc.Bacc("TRN2", target_bir_lowering=False, debug=False, num_devices=C)
    NP_ = ntiles_pc * P

    d = lambda name, shape, dt=F32, kind="ExternalInput": nc.dram_tensor(
        name, shape, dt, kind=kind
    ).ap()

    st_d = d("st", [(SL // P) * 3 * D, P], FP8)
    ownT_d = d("own_nodesT", [ntiles_pc * D, P])
    wtb = d("wtb", [D, 2 * D])  # [wt*WSC | wb*WSC]
    mb8 = d("mb8", [P, 8 * D])  # mb1*WSC tiled
    ua = d("ua", [D, D])
    ub = d("ub", [D, D])
    uw2 = d("uw2", [D, D])
    ub1c = d("ub1c", [P, 1])
    ub2c = d("ub2c", [P, 1])
    ident = d("ident", [P, P])
    out = d("out_own", [NP_, D], kind="ExternalOutput")

    with tile.TileContext(nc) as tc:
        with (
            tc.tile_pool(name="const", bufs=1) as cp,
            tc.tile_pool(name="sb", bufs=5) as sb,
            tc.tile_pool(name="big", bufs=4) as bigp,
            tc.tile_pool(name="psum2", bufs=2, space="PSUM") as pp,
            tc.tile_pool(name="psumM", bufs=3, space="PSUM") as ppm,
            tc.tile_pool(name="psumA", bufs=3, space="PSUM") as ppa,
        ):
            def load_const(ap, shape, dt=F32):
                t = cp.tile(shape, dt, tag=ap.name)
                nc.sync.dma_start(out=t[:], in_=ap[:])
                return t

            wtb_s = load_const(wtb, [D, 2 * D])
            mb8_s = load_const(mb8, [P, 8 * D])
            ua_s = load_const(ua, [D, D])
            ub_s = load_const(ub, [D, D])
            uw2_s = load_const(uw2, [D, D])
            ub1_s = load_const(ub1c, [P, 1])
            ub2_s = load_const(ub2c, [P, 1])
            id_s = load_const(ident, [P, P])
            aggT_all = cp.tile([P, ntiles_pc * D], F32, tag="aggT_all")
            wtb8 = cp.tile([D, 2 * D], FP8, tag="wtb8")
            nc.vector.tensor_copy(out=wtb8[:], in_=wtb_s[:])
            wtb8_r = wtb8[:].rearrange("p (two f) -> p two f", two=2)

            def update_group(g):
                """Stage 3 for node tiles [4g, 4g+4): update MLP + residual."""
                gw = min(4, ntiles_pc - g * 4)
                W = gw * P
                g0 = g * 4
                ownT = sb.tile([P, 4 * P], F32, tag="ownT")
                nc.sync.dma_start(
                    out=ownT[:, :W].rearrange("p (j n) -> p j n", n=P),
                    in_=ownT_d[g0 * D : (g0 + gw) * D, :].rearrange(
                        "(j d) n -> d j n", d=D
                    ),
                )
                ph = pp.tile([P, 4 * P], F32, tag="ph")
                nc.tensor.matmul(
                    out=ph[:, :W], lhsT=ua_s[:], rhs=ownT[:, :W], start=True,
                    stop=False,
                )
                nc.tensor.matmul(
                    out=ph[:, :W],
                    lhsT=ub_s[:],
                    rhs=aggT_all[:, g0 * D : g0 * D + W],
                    start=False,
                    stop=True,
                )
                hT = sb.tile([P, 4 * P], F32, tag="hT")
                nc.scalar.activation(
                    out=hT[:, :W], in_=ph[:, :W], func=AF.Silu, bias=ub1_s[:, :1]
                )
                po = pp.tile([P, 4 * P], F32, tag="ph")
                nc.tensor.matmul(
                    out=po[:, :W], lhsT=uw2_s[:], rhs=hT[:, :W], start=True, stop=True
                )
                oT = sb.tile([P, 4 * P], F32, tag="oT")
                nc.scalar.activation(
                    out=oT[:, :W], in_=po[:, :W], func=AF.Identity, bias=ub2_s[:, :1]
                )
                nc.vector.tensor_tensor(
                    out=oT[:, :W], in0=oT[:, :W], in1=ownT[:, :W], op=OP.add
                )
                pOut = pp.tile([P, 4 * P], F32, tag="ph")
                for j in range(gw):
                    nc.tensor.transpose(
                        out=pOut[:, j * P : (j + 1) * P],
                        in_=oT[:, j * P : (j + 1) * P],
                        identity=id_s[:],
                    )
                ot = sb.tile([P, 4 * P], F32, tag="ot")
                nc.vector.tensor_copy(out=ot[:, :W], in_=pOut[:, :W])
                nc.sync.dma_start(
                    out=out[g0 * P : (g0 + gw) * P, :].rearrange(
                        "(j p) d -> p j d", p=P
                    ),
                    in_=ot[:, :W].rearrange("p (j d) -> p j d", d=D),
                )

            # empty (pure-pad) tiles never write aggT_all; clear once
            nc.vector.memset(aggT_all[:], 0)

            for t in range(ntiles_pc):
                kt = ktot[t]
                if kt:
                    egT = bigp.tile([P, 3 * kt * D], FP8, tag="egT")
                    eng = nc.scalar if t % 2 == 0 else nc.sync
                    eng.dma_start(
                        out=egT[:].rearrange("p (k e) -> p k e", e=P),
                        in_=st_d[offs[t] * 3 * D : offs[t + 1] * 3 * D, :].rearrange(
                            "(k d) e -> d k e", d=D
                        ),
                    )
                    LAG = 2  # scatter mms trail msg mms by this many chunks
                    paggT = ppa.tile([P, D], F32, tag="paggT")

                    def scatter(k0, cw, msg):
                        for j in range(cw):
                            k = k0 + j
                            # aggT[d, j] += msg_k^T-contraction over e
                            nc.tensor.matmul(
                                out=paggT[:],
                                lhsT=msg[:, j * P : (j + 1) * P],
                                rhs=egT[:, ((k * 3) + 2) * D : ((k * 3) + 3) * D],
                                start=(k == 0),
                                stop=(k == kt - 1),
                            )

                    pend = []
                    for ci in range(math.ceil(kt / 4)):
                        k0 = ci * 4
                        cw = min(4, kt - k0)
                        W = cw * P
                        pmsg = ppm.tile([P, 4 * P], F32, tag="pmsg")
                        for j in range(cw):
                            o = (k0 + j) * 3
                            nc.tensor.matmul(
                                out=pmsg[:, j * P : (j + 1) * P],
                                lhsT=egT[:, o * D : (o + 2) * D].rearrange(
                                    "p (two e) -> p two e", two=2
                                ),
                                rhs=wtb8_r,
                                start=True,
                                stop=True,
                                perf_mode=mybir.MatmulPerfMode.DoubleRow,
                            )
                        nc.vector.tensor_tensor(
                            out=pmsg[:, :W],
                            in0=pmsg[:, :W],
                            in1=mb8_s[:, :W],
                            op=OP.add,
                        )
                        msg = sb.tile([P, 4 * P], BF16, tag="msg")
                        nc.scalar.activation(
                            out=msg[:, :W],
                            in_=pmsg[:, :W],
                            func=AF.Silu,
                            scale=1.0 / WSC,
                        )
                        pend.append((k0, cw, msg))
                        if len(pend) > LAG:
                            scatter(*pend.pop(0))
                    for args in pend:
                        scatter(*args)
                    nc.vector.tensor_copy(
                        out=aggT_all[:, t * D : (t + 1) * D], in_=paggT[:]
                    )
                if t % 4 == 3:
                    update_group(t // 4)
            if ntiles_pc % 4:
                update_group(ntiles_pc // 4)

    nc.compile()
    return nc


def _run(nc, in_maps, trace=False):
    return bass_utils.run_bass_kernel_spmd(
        nc, in_maps, core_ids=list(range(C)), trace=trace
    )


def make_in_maps(nodes, edge_index, edge_features, mw1, mb1, uw1, ub1, uw2, ub2,
                 ntiles_pc):
    N, D = nodes.shape
    NP_ = ntiles_pc * P
    N2 = NP_ * C
    ktot, assign, per_core = _host_prep(nodes, edge_index, edge_features, ntiles_pc)

    nodes_pad = np.zeros((N2, D), np.float32)
    nodes_pad[:N] = nodes
    ident = np.eye(P, dtype=np.float32)
    mb8 = np.broadcast_to(
        np.tile(mb1.astype(np.float32) * WSC, 8), (P, 8 * D)
    ).copy()

    shared = dict(
        wtb=np.concatenate(
            [mw1[:D] * WSC, mw1[D:] * WSC], axis=1
        ).astype(np.float32),
        mb8=mb8,
        ua=np.ascontiguousarray(uw1[:D], np.float32),
        ub=np.ascontiguousarray(uw1[D:], np.float32),
        uw2=np.ascontiguousarray(uw2, np.float32),
        ub1c=np.ascontiguousarray(ub1.reshape(D, 1), np.float32),
        ub2c=np.ascontiguousarray(ub2.reshape(D, 1), np.float32),
        ident=ident,
    )
    in_maps = []
    for c in range(C):
        m = dict(shared)
        own = np.concatenate(
            [nodes_pad[g * P : (g + 1) * P] for g in assign[c]], axis=0
        )
        m["own_nodesT"] = _tileT(np.ascontiguousarray(own))
        m["st"] = per_core[c]["st"]
        in_maps.append(m)
    return ktot, assign, in_maps


def kernel(nodes, edge_index, edge_features, mw1, mb1, uw1, ub1, uw2, ub2):
    nodes = np.asarray(nodes, np.float32)
    edge_index = np.asarray(edge_index, np.int32)
    edge_features = np.asarray(edge_features, np.float32)
    N, D = nodes.shape
    ntiles_pc = math.ceil(N / (C * P))
    ktot, assign, in_maps = make_in_maps(
        nodes, edge_index, edge_features, mw1, mb1, uw1, ub1, uw2, ub2, ntiles_pc
    )
    N2 = ntiles_pc * P * C
    nc = build_program(N2, D, ntiles_pc, ktot)
    res = _run(nc, in_maps)
    out_full = np.zeros((N2, D), np.float32)
    for c in range(C):
        oc = res.results[c]["out_own"]
        for p, g in enumerate(assign[c]):
            out_full[g * P : (g + 1) * P] = oc[p * P : (p + 1) * P]
    return out_full[:N].astype(np.float32)


if __name__ == "__main__":
    rng = np.random.default_rng(0)
    N, E, D = 4096, 16384, 128
    nodes = rng.standard_normal((N, D), dtype=np.float32)
    edge_index = rng.integers(0, N, (2, E)).astype(np.int32)
    ef = rng.standard_normal((E, D), dtype=np.float32)
    s2, s1 = 1 / np.sqrt(2 * D), 1 / np.sqrt(D)
    mw1 = rng.uniform(-s2, s2, (2 * D, D)).astype(np.float32)
    mb1 = rng.uniform(-s2, s2, D).astype(np.float32)
    uw1 = rng.uniform(-s2, s2, (2 * D, D)).astype(np.float32)
    ub1 = rng.uniform(-s2, s2, D).astype(np.float32)
    uw2 = rng.uniform(-s1, s1, (D, D)).astype(np.float32)
    ub2 = rng.uniform(-s1, s1, D).astype(np.float32)

    def silu(x):
        return x / (1 + np.exp(-x))

    def ref():
        src, dst = edge_index
        msg = silu(np.concatenate([nodes[src], ef], 1) @ mw1 + mb1)
        agg = np.zeros((N, D), np.float32)
        np.add.at(agg, dst, msg)
        upd = silu(np.concatenate([nodes, agg], 1) @ uw1 + ub1) @ uw2 + ub2
        return nodes + upd

    out = kernel(nodes, edge_index, ef, mw1, mb1, uw1, ub1, uw2, ub2)
    exp = ref()
    err = np.abs(out - exp).max() / np.abs(exp).max()
    print("tiny rel err:", err)
